# revision 11
# baseline (speedup 1.0000x reference)
"""CrossAttention (B=4, N=M=2048, C=1024, H=16, D=64) on 8 TRN2 cores.

Sharding: core = 2*b + g  (b = batch 0..3, g = head-half 0..1, 8 heads each).
Each core computes attention for its 8 heads and a partial (full-width)
output projection over its 512 local channels; the host sums the two
partials per batch and transposes back.

Device layout notes:
  - All activations live transposed (channels on partitions) so every
    matmul has its contraction on the partition axis with no on-chip
    transposes.  The host feeds query/key/value pre-transposed and the
    weights pre-tiled to the exact SBUF layout (contiguous DMAs).
  - The whole attention phase runs in the PE's 64x128 row-tiled mode
    (2 concurrent half-array tiles T0/T8), with no mode switches:
      * scoresT per head pair: head 2j on tile (0,0), head 2j+1 on
        (64,0), writing the two 512-wide halves (different PSUM banks)
        of one [128,1024] tile -> one 1024-wide exp covers both heads.
      * AV is split over key halves: keys 0-63 of a tile on (0,0),
        keys 64-127 on (64,0), accumulated into separate lo/hi PSUM
        accumulators that a DVE add folds together at the end.
    Mixing 64-row scores with 128-row AV (the obvious layout) would
    drain the PE array on every row-tiling mode switch (~2 per key
    tile, ~512 per kernel).
  - softmax denominator comes free from a ones-column appended to V
    (65-col AV stationary; row 64 of the accumulator is sum_m exp(s)).
  - exp on the scalar engine (psum->sbuf, width 1024, shifted by -2 so
    bf16 p~ stays in range; softmax is shift-invariant).
  - Normalization: DVE lo+hi fold + reciprocal of the denominator row +
    GPSIMD partition-broadcast + DVE multiply into xT.
  - Projection order q, k, v lets the score/exp stream start while v
    still projects (scores only need qT/kT + psum).
"""

from contextlib import ExitStack

import ml_dtypes
import numpy as np

import concourse.bass as bass
import concourse.mybir as mybir
import concourse.tile as tile
from concourse import bacc, library_config
from concourse.bass_utils import run_bass_kernel_spmd

dt = mybir.dt
AF = mybir.ActivationFunctionType

# Problem dims (hardcoded; must match the harness inputs).
B, N, M, C, H = 4, 2048, 2048, 1024, 16
D = C // H            # 64
SCALE = D ** -0.5     # 0.125 (exact)
CL = C // 2           # 512 channels per core (8 heads)
HL = H // 2           # 8 local heads
P = 128
CT = C // P           # 8 input-channel tiles
DT = CL // P          # 4 local-channel tiles
MT = M // P           # 16 key tiles
NCH = 512             # psum bank width in fp32
NCHUNKS = N // NCH    # 4
EXPW = 1024           # exp width (2 psum banks)
VA = D + 1            # 65: v columns + ones column

F32 = dt.float32
F32R = dt.float32r
BF16 = dt.bfloat16


def build_program(reps: int = 1, mode: str = "") -> bass.Bass:
    """reps>1 repeats the whole body for timing (wall-time delta isolates
    device time from host/transfer overhead).

    mode flags (diagnostics): 'P' stop after projections, 'A' stop after
    attention, 'O' skip attention (zero xT), 'X' skip final output DMA."""
    nc = bacc.Bacc()
    nc.gpsimd.load_library(library_config.attn)

    qTin = nc.declare_dram_parameter("qTin", [C, N], F32R, isOutput=False)
    kTin = nc.declare_dram_parameter("kTin", [C, M], F32R, isOutput=False)
    vTin = nc.declare_dram_parameter("vTin", [C, M], F32R, isOutput=False)
    # weights arrive pre-tiled to SBUF layout: contiguous (P, x) DMAs
    wq = nc.declare_dram_parameter("wq", [P, CT * CL], F32R, isOutput=False)
    wk = nc.declare_dram_parameter("wk", [P, CT * CL], F32R, isOutput=False)
    wv = nc.declare_dram_parameter("wv", [P, CT * CL], F32R, isOutput=False)
    wp = nc.declare_dram_parameter("wp", [P, DT * C], BF16, isOutput=False)
    bp = nc.declare_dram_parameter("bp", [P, CT], F32, isOutput=False)
    out = nc.declare_dram_parameter("out", [C, N], F32, isOutput=True)

    with tile.TileContext(nc) as tc:
      for _rep in range(reps):
       with ExitStack() as ctx:
        # ---- persistent sbuf tensors -------------------------------------
        const_pool = ctx.enter_context(tc.tile_pool(name="consts", bufs=1))
        bp_sb = const_pool.tile([P, CT], F32)
        qT_sb = const_pool.tile([P, DT * N], F32R)   # local q, transposed
        kT_sb = const_pool.tile([P, DT * M], F32R)   # local k, transposed
        # v(+ones) per key tile and head: [P][mt][h][VA]
        va_sb = const_pool.tile([P, MT * HL * VA], BF16)
        xT_sb = const_pool.tile([P, DT * N], BF16)  # attention out, transposed

        nc.sync.dma_start(out=bp_sb[:], in_=bp[:, :])

        va3 = va_sb[:].rearrange("p (m h e) -> p m h e", m=MT, h=HL, e=VA)

        # ---- attention pools; pt opens before phase 1 so exp can run
        #      while v still projects (scores only need qT/kT + psum) ----
        pt_pool = ctx.enter_context(tc.tile_pool(name="pt", bufs=12))

        # ---- phase 1: projections (q, k halved; v with all tiles) ------
        with tc.tile_pool(name="inT", bufs=8) as in_pool, \
             tc.tile_pool(name="wcur", bufs=1) as w_pool, \
             tc.tile_pool(name="ps2", bufs=2, space="PSUM") as ps_pool:

            def load_ctile(src, ct):
                t = in_pool.tile([P, N], F32R, tag="inT", name=f"i{ct}")
                nc.sync.dma_start(out=t[:], in_=src[ct * P:(ct + 1) * P, :])
                return t

            # --- q and k projections (halved accumulation) ---
            for src, w_dram, dst_sb in ((qTin, wq, qT_sb), (kTin, wk, kT_sb)):
                w_sb = w_pool.tile([P, CT * CL], F32R, tag="w", name="w")
                nc.sync.dma_start(out=w_sb[:], in_=w_dram[:, :])
                for half in range(2):
                    tiles = [load_ctile(src, half * 4 + ci) for ci in range(4)]
                    for j in range(DT):
                        for ch in range(NCHUNKS):
                            acc = ps_pool.tile([P, EXPW], F32, tag="big",
                                               name="prj")
                            for ci in range(4):
                                ct = half * 4 + ci
                                nc.tensor.matmul(
                                    acc[:, :NCH],
                                    w_sb[:, ct * CL + j * P: ct * CL + (j + 1) * P],
                                    tiles[ci][:, ch * NCH:(ch + 1) * NCH],
                                    start=(ci == 0),
                                    stop=(ci == 3),
                                )
                            dst = dst_sb[:, j * N + ch * NCH:
                                         j * N + (ch + 1) * NCH]
                            if half == 0:
                                nc.vector.tensor_copy(dst, acc[:, :NCH])
                            else:
                                nc.vector.tensor_add(dst, acc[:, :NCH], dst)

            # --- v projection (natural orientation, all c-tiles live) ---
            wv_sb = w_pool.tile([P, CT * CL], F32R, tag="w", name="w")
            nc.sync.dma_start(out=wv_sb[:], in_=wv[:, :])
            vtiles = [load_ctile(vTin, ct) for ct in range(CT)]
            for mt in range(MT):
                acc = ps_pool.tile([P, EXPW], F32, tag="big", name="prv")
                for ct in range(CT):
                    nc.tensor.matmul(
                        acc[:, :CL],
                        vtiles[ct][:, mt * P:(mt + 1) * P],
                        wv_sb[:, ct * CL:(ct + 1) * CL],
                        start=(ct == 0),
                        stop=(ct == CT - 1),
                    )
                blk = va3[:, mt, :, :]                    # (P, HL, VA)
                nc.vector.tensor_copy(
                    blk[:, :, :D],
                    acc[:, :CL].rearrange("p (h d) -> p h d", d=D),
                )
                nc.vector.memset(blk[:, :, D:VA], 1.0)

        if "P" in mode:
            continue

        # late pools reuse the freed phase-1 space
        sc_pool = ctx.enter_context(tc.tile_pool(name="sc", bufs=4, space="PSUM"))
        av_pool = ctx.enter_context(tc.tile_pool(name="av", bufs=4, space="PSUM"))
        wpx_pool = ctx.enter_context(tc.tile_pool(name="wpx", bufs=1))
        sm_pool = ctx.enter_context(tc.tile_pool(name="sm", bufs=4))
        ob_pool = ctx.enter_context(tc.tile_pool(name="ob", bufs=3))
        wp_sb = wpx_pool.tile([P, DT * C], BF16)
        nc.sync.dma_start(out=wp_sb[:], in_=wp[:, :])

        if "O" in mode:
            nc.vector.memset(xT_sb[:], 0.0)

        # ---- phase 2: attention, head pair x 512-query chunks, all in
        #      64x128 row-tiled mode ----------------------------------
        for j in ([] if "O" in mode else range(DT)):   # head pair j: 2j, 2j+1
            for nh in range(NCHUNKS):                  # 512-query chunk
                q0 = j * N + nh * NCH
                # lo/hi key-half accumulators per head: [hh][half]
                avs = [
                    [av_pool.tile([P, NCH], F32, tag="av",
                                  name=f"av{hh}_{half}")
                     for half in range(2)]
                    for hh in range(2)
                ]
                pend = None

                def emit_av(pts, mt):
                    for hh in range(2):
                        h = 2 * j + hh
                        for half in range(2):
                            po = half * D
                            nc.tensor.matmul(
                                avs[hh][half][:VA, :],
                                va3[po:po + D, mt, h, :],
                                pts[hh][po:po + D, :],
                                start=(mt == 0),
                                stop=(mt == MT - 1),
                            )

                for mt in range(MT):
                    scs = [sc_pool.tile([P, NCH], F32, tag="sc", name="sc")
                           for _ in range(2)]
                    for hh in range(2):
                        roff = hh * D
                        nc.tensor.matmul(
                            scs[hh][:, :],
                            kT_sb[roff:roff + D,
                                  j * M + mt * P: j * M + (mt + 1) * P],
                            qT_sb[roff:roff + D, q0:q0 + NCH],
                            start=True,
                            stop=True,
                        )
                    pts = []
                    for hh in range(2):
                        pt = pt_pool.tile([P, NCH], BF16, tag="pt", name="pt")
                        nc.scalar.activation(pt[:], scs[hh][:], AF.Exp)
                        pts.append(pt)
                    if pend is not None:
                        emit_av(*pend)
                    pend = (pts, mt)
                emit_av(*pend)

                for hh in range(2):
                    roff = hh * D
                    # fold lo+hi key-half accumulators (DVE reads at most
                    # one PSUM operand per op: copy then in-place add)
                    av = sm_pool.tile([VA, NCH], F32, tag="avf", name="avf")
                    nc.vector.tensor_copy(av[:], avs[hh][1][:VA, :])
                    nc.vector.tensor_add(av[:], avs[hh][0][:VA, :], av[:])
                    rc = sm_pool.tile([1, NCH], F32, tag="rc", name="rc")
                    nc.vector.reciprocal(rc[:], av[D:VA, :])
                    bc = sm_pool.tile([D, NCH], F32, tag="bc", name="bc")
                    nc.gpsimd.partition_broadcast(bc[:], rc[:])
                    nc.vector.tensor_mul(
                        xT_sb[roff:roff + D, q0:q0 + NCH],
                        av[:D, :],
                        bc[:],
                    )

        if "A" in mode:
            continue
        # ---- phase 3: output projection (partial over local channels) ---
        for mt8 in range(CT):
            ob = ob_pool.tile([P, N], F32, tag="ob", name="ob")
            for ch in range(NCHUNKS):
                acc = sc_pool.tile([P, NCH], F32, tag="sc", name="po")
                for ct in range(DT):
                    nc.tensor.matmul(
                        acc[:, :NCH],
                        wp_sb[:, ct * C + mt8 * P: ct * C + (mt8 + 1) * P],
                        xT_sb[:, ct * N + ch * NCH: ct * N + (ch + 1) * NCH],
                        start=(ct == 0),
                        stop=(ct == DT - 1),
                    )
                nc.vector.tensor_scalar_add(
                    ob[:, ch * NCH:(ch + 1) * NCH], acc[:, :NCH],
                    bp_sb[:, mt8:mt8 + 1])
            if "X" not in mode:
                nc.sync.dma_start(out=out[mt8 * P:(mt8 + 1) * P, :], in_=ob[:])

    nc.compile()
    return nc


_NC_CACHE = {}


def _get_program(reps: int = 1, mode: str = ""):
    key = (reps, mode)
    if key not in _NC_CACHE:
        _NC_CACHE[key] = build_program(reps, mode)
    return _NC_CACHE[key]


def _tile_w(wT_slice):
    """(C, CL) weight slice -> pre-tiled (P, CT*CL) SBUF image."""
    c, cl = wT_slice.shape
    return np.ascontiguousarray(
        wT_slice.reshape(c // P, P, cl).transpose(1, 0, 2).reshape(P, -1))


def make_in_maps(query, key, value, Wq, Wk, Wv, Wp, bp):
    query = np.asarray(query, dtype=np.float32)
    key = np.asarray(key, dtype=np.float32)
    value = np.asarray(value, dtype=np.float32)
    Wq = np.asarray(Wq, dtype=np.float32)
    Wk = np.asarray(Wk, dtype=np.float32)
    Wv = np.asarray(Wv, dtype=np.float32)
    Wp = np.asarray(Wp, dtype=np.float32)
    bp = np.asarray(bp, dtype=np.float32)

    wqT = np.ascontiguousarray(Wq.T) * np.float32(SCALE)  # (C, C)
    wkT = np.ascontiguousarray(Wk.T)
    wvT = np.ascontiguousarray(Wv.T)
    wpT = np.ascontiguousarray(Wp.T)                      # (C, C)
    zeros_bp = np.zeros_like(bp)

    in_maps = []
    for core in range(8):
        b, g = divmod(core, 2)
        sl = slice(g * CL, (g + 1) * CL)
        bpc = (bp if g == 0 else zeros_bp)
        in_maps.append({
            "qTin": np.ascontiguousarray(query[b].T),
            "kTin": np.ascontiguousarray(key[b].T),
            "vTin": np.ascontiguousarray(value[b].T),
            "wq": _tile_w(wqT[:, sl]),
            "wk": _tile_w(wkT[:, sl]),
            "wv": _tile_w(wvT[:, sl]),
            "wp": _tile_w(wpT[sl, :]).astype(ml_dtypes.bfloat16),
            "bp": np.ascontiguousarray(bpc.reshape(CT, P).T),
        })
    return in_maps


def combine_outputs(results):
    out = np.empty((B, N, C), dtype=np.float32)
    for b in range(B):
        part = results[2 * b]["out"] + results[2 * b + 1]["out"]  # (C, N)
        out[b] = part.T
    return out


def kernel(**inputs) -> np.ndarray:
    nc = _get_program()
    in_maps = make_in_maps(**inputs)
    res = run_bass_kernel_spmd(nc, in_maps, list(range(8)))
    return combine_outputs(res.results)


if __name__ == "__main__":
    nc = _get_program()
    print("program built ok")


# revision 15
# speedup vs baseline: 1.0114x; 1.0114x over previous
"""CrossAttention (B=4, N=M=2048, C=1024, H=16, D=64) on 8 TRN2 cores.

Sharding: core = 2*b + g  (b = batch 0..3, g = head-half 0..1, 8 heads each).
Each core computes attention for its 8 heads and a partial (full-width)
output projection over its 512 local channels; the host sums the two
partials per batch and transposes back.

Device layout notes:
  - All activations live transposed (channels on partitions) so every
    matmul has its contraction on the partition axis with no on-chip
    transposes.  The host feeds query/key/value pre-transposed and the
    weights pre-tiled to the exact SBUF layout (contiguous DMAs).
  - The whole attention phase runs in the PE's 64x128 row-tiled mode
    (2 concurrent half-array tiles T0/T8), with no mode switches:
      * scoresT per head pair: head 2j on tile (0,0), head 2j+1 on
        (64,0), writing the two 512-wide halves (different PSUM banks)
        of one [128,1024] tile -> one 1024-wide exp covers both heads.
      * AV is split over key halves: keys 0-63 of a tile on (0,0),
        keys 64-127 on (64,0), accumulated into separate lo/hi PSUM
        accumulators that a DVE add folds together at the end.
    Mixing 64-row scores with 128-row AV (the obvious layout) would
    drain the PE array on every row-tiling mode switch (~2 per key
    tile, ~512 per kernel).
  - softmax denominator comes free from a ones-column appended to V
    (65-col AV stationary; row 64 of the accumulator is sum_m exp(s)).
  - exp on the scalar engine (psum->sbuf, width 1024, shifted by -2 so
    bf16 p~ stays in range; softmax is shift-invariant).
  - Normalization: DVE lo+hi fold + reciprocal of the denominator row +
    GPSIMD partition-broadcast + DVE multiply into xT.
  - Projection order q, k, v lets the score/exp stream start while v
    still projects (scores only need qT/kT + psum).
"""

from contextlib import ExitStack

import ml_dtypes
import numpy as np

import concourse.bass as bass
import concourse.mybir as mybir
import concourse.tile as tile
from concourse import bacc, library_config
from concourse.bass_utils import run_bass_kernel_spmd

dt = mybir.dt
AF = mybir.ActivationFunctionType

# Problem dims (hardcoded; must match the harness inputs).
B, N, M, C, H = 4, 2048, 2048, 1024, 16
D = C // H            # 64
SCALE = D ** -0.5     # 0.125 (exact)
CL = C // 2           # 512 channels per core (8 heads)
HL = H // 2           # 8 local heads
P = 128
CT = C // P           # 8 input-channel tiles
DT = CL // P          # 4 local-channel tiles
MT = M // P           # 16 key tiles
NCH = 512             # psum bank width in fp32
NCHUNKS = N // NCH    # 4
EXPW = 1024           # exp width (2 psum banks)
VA = D + 1            # 65: v columns + ones column

F32 = dt.float32
F32R = dt.float32r
BF16 = dt.bfloat16


def build_program(reps: int = 1, mode: str = "") -> bass.Bass:
    """reps>1 repeats the whole body for timing (wall-time delta isolates
    device time from host/transfer overhead).

    mode flags (diagnostics): 'P' stop after projections, 'A' stop after
    attention, 'O' skip attention (zero xT), 'X' skip final output DMA."""
    nc = bacc.Bacc()
    nc.gpsimd.load_library(library_config.attn)

    qTin = nc.declare_dram_parameter("qTin", [C, N], F32R, isOutput=False)
    kTin = nc.declare_dram_parameter("kTin", [C, M], F32R, isOutput=False)
    vTin = nc.declare_dram_parameter("vTin", [C, M], F32R, isOutput=False)
    # weights arrive pre-tiled to SBUF layout: contiguous (P, x) DMAs
    wq = nc.declare_dram_parameter("wq", [P, CT * CL], F32R, isOutput=False)
    wk = nc.declare_dram_parameter("wk", [P, CT * CL], F32R, isOutput=False)
    wv = nc.declare_dram_parameter("wv", [P, CT * CL], F32R, isOutput=False)
    wp = nc.declare_dram_parameter("wp", [P, DT * C], BF16, isOutput=False)
    bp = nc.declare_dram_parameter("bp", [P, CT], F32, isOutput=False)
    out = nc.declare_dram_parameter("out", [C, N], F32, isOutput=True)

    with tile.TileContext(nc) as tc:
      for _rep in range(reps):
       with ExitStack() as ctx:
        # ---- persistent sbuf tensors -------------------------------------
        const_pool = ctx.enter_context(tc.tile_pool(name="consts", bufs=1))
        bp_sb = const_pool.tile([P, CT], F32)
        # q in two zero-padded copies: qTp[0] has head 2j's dims in rows
        # 0-63 and zeros in 64-127, qTp[1] the reverse.  Both score
        # matmuls of a head pair can then share the one full-128-row kT
        # stationary (the zero rows kill the other head's contraction),
        # which keeps the whole kernel in the PE's 128-row mode: no
        # row-tiling mode-switch drains, one ldweights per key tile.
        qTp = [const_pool.tile([P, DT * N], BF16, name=f"qTp{i}")
               for i in range(2)]
        kT_sb = const_pool.tile([P, DT * M], BF16)   # local k, transposed
        # v(+ones) per key tile and head: [P][mt][h][VA]
        va_sb = const_pool.tile([P, MT * HL * VA], BF16)
        xT_sb = const_pool.tile([P, DT * N], BF16)  # attention out, transposed

        nc.sync.dma_start(out=bp_sb[:], in_=bp[:, :])
        nc.vector.memset(qTp[0][D:P, :], 0.0)
        nc.vector.memset(qTp[1][0:D, :], 0.0)

        va3 = va_sb[:].rearrange("p (m h e) -> p m h e", m=MT, h=HL, e=VA)

        # ---- attention pools; pt opens before phase 1 so exp can run
        #      while v still projects (scores only need qT/kT + psum) ----
        pt_pool = ctx.enter_context(tc.tile_pool(name="pt", bufs=12))

        # ---- phase 1: projections (q, k halved; v with all tiles) ------
        with tc.tile_pool(name="inT", bufs=8) as in_pool, \
             tc.tile_pool(name="wcur", bufs=1) as w_pool, \
             tc.tile_pool(name="ps2", bufs=2, space="PSUM") as ps_pool:

            def load_ctile(src, ct):
                t = in_pool.tile([P, N], F32R, tag="inT", name=f"i{ct}")
                nc.sync.dma_start(out=t[:], in_=src[ct * P:(ct + 1) * P, :])
                return t

            # --- q and k projections (halved accumulation) ---
            for src, w_dram, dsts in ((qTin, wq, None), (kTin, wk, kT_sb)):
                w_sb = w_pool.tile([P, CT * CL], F32R, tag="w", name="w")
                nc.sync.dma_start(out=w_sb[:], in_=w_dram[:, :])
                for half in range(2):
                    tiles = [load_ctile(src, half * 4 + ci) for ci in range(4)]
                    for j in range(DT):
                        for ch in range(NCHUNKS):
                            acc = ps_pool.tile([P, EXPW], F32, tag="big",
                                               name="prj")
                            for ci in range(4):
                                ct = half * 4 + ci
                                nc.tensor.matmul(
                                    acc[:, :NCH],
                                    w_sb[:, ct * CL + j * P: ct * CL + (j + 1) * P],
                                    tiles[ci][:, ch * NCH:(ch + 1) * NCH],
                                    start=(ci == 0),
                                    stop=(ci == 3),
                                )
                            col = slice(j * N + ch * NCH,
                                        j * N + (ch + 1) * NCH)
                            if dsts is None:
                                # q: split the head pair into the padded
                                # copies (rows 0-63 -> qTp[0], 64-127 -> qTp[1])
                                parts = [(qTp[0][0:D, col], acc[0:D, :NCH]),
                                         (qTp[1][D:P, col], acc[D:P, :NCH])]
                            else:
                                parts = [(dsts[:, col], acc[:, :NCH])]
                            for dst, s_ in parts:
                                if half == 0:
                                    nc.vector.tensor_copy(dst, s_)
                                else:
                                    nc.vector.tensor_add(dst, s_, dst)

            # --- v projection (natural orientation, all c-tiles live) ---
            wv_sb = w_pool.tile([P, CT * CL], F32R, tag="w", name="w")
            nc.sync.dma_start(out=wv_sb[:], in_=wv[:, :])
            vtiles = [load_ctile(vTin, ct) for ct in range(CT)]
            for mt in range(MT):
                acc = ps_pool.tile([P, EXPW], F32, tag="big", name="prv")
                for ct in range(CT):
                    nc.tensor.matmul(
                        acc[:, :CL],
                        vtiles[ct][:, mt * P:(mt + 1) * P],
                        wv_sb[:, ct * CL:(ct + 1) * CL],
                        start=(ct == 0),
                        stop=(ct == CT - 1),
                    )
                blk = va3[:, mt, :, :]                    # (P, HL, VA)
                nc.vector.tensor_copy(
                    blk[:, :, :D],
                    acc[:, :CL].rearrange("p (h d) -> p h d", d=D),
                )
                nc.vector.memset(blk[:, :, D:VA], 1.0)

        if "P" in mode:
            continue

        # late pools reuse the freed phase-1 space
        sc_pool = ctx.enter_context(tc.tile_pool(name="sc", bufs=4, space="PSUM"))
        av_pool = ctx.enter_context(tc.tile_pool(name="av", bufs=4, space="PSUM"))
        wpx_pool = ctx.enter_context(tc.tile_pool(name="wpx", bufs=1))
        sm_pool = ctx.enter_context(tc.tile_pool(name="sm", bufs=4))
        ob_pool = ctx.enter_context(tc.tile_pool(name="ob", bufs=3))
        wp_sb = wpx_pool.tile([P, DT * C], BF16)
        nc.sync.dma_start(out=wp_sb[:], in_=wp[:, :])

        if "O" in mode:
            nc.vector.memset(xT_sb[:], 0.0)

        # ---- phase 2: attention, head pair x 512-query chunks, all in
        #      64x128 row-tiled mode ----------------------------------
        for j in ([] if "O" in mode else range(DT)):   # head pair j: 2j, 2j+1
            for nh in range(NCHUNKS):                  # 512-query chunk
                q0 = j * N + nh * NCH
                avs = [av_pool.tile([P, NCH], F32, tag="av", name=f"av{hh}")
                       for hh in range(2)]
                pend = None

                def emit_av(pts, mt):
                    for hh in range(2):
                        nc.tensor.matmul(
                            avs[hh][:VA, :],
                            va3[:, mt, 2 * j + hh, :],
                            pts[hh][:, :],
                            start=(mt == 0),
                            stop=(mt == MT - 1),
                        )

                for mt in range(MT):
                    kst = kT_sb[:, j * M + mt * P: j * M + (mt + 1) * P]
                    scs = [sc_pool.tile([P, NCH], F32, tag="sc", name="sc")
                           for _ in range(2)]
                    for hh in range(2):
                        nc.tensor.matmul(
                            scs[hh][:, :], kst, qTp[hh][:, q0:q0 + NCH],
                            start=True, stop=True,
                        )
                    pts = []
                    for hh in range(2):
                        pt = pt_pool.tile([P, NCH], BF16, tag="pt", name="pt")
                        nc.scalar.activation(pt[:], scs[hh][:], AF.Exp)
                        pts.append(pt)
                    if pend is not None:
                        emit_av(*pend)
                    pend = (pts, mt)
                emit_av(*pend)

                for hh in range(2):
                    roff = hh * D
                    rc = sm_pool.tile([1, NCH], F32, tag="rc", name="rc")
                    nc.vector.reciprocal(rc[:], avs[hh][D:VA, :])
                    bc = sm_pool.tile([D, NCH], F32, tag="bc", name="bc")
                    nc.gpsimd.partition_broadcast(bc[:], rc[:])
                    nc.vector.tensor_mul(
                        xT_sb[roff:roff + D, q0:q0 + NCH],
                        avs[hh][:D, :],
                        bc[:],
                    )

        if "A" in mode:
            continue
        # ---- phase 3: output projection (partial over local channels) ---
        for mt8 in range(CT):
            ob = ob_pool.tile([P, N], F32, tag="ob", name="ob")
            for ch in range(NCHUNKS):
                acc = sc_pool.tile([P, NCH], F32, tag="sc", name="po")
                for ct in range(DT):
                    nc.tensor.matmul(
                        acc[:, :NCH],
                        wp_sb[:, ct * C + mt8 * P: ct * C + (mt8 + 1) * P],
                        xT_sb[:, ct * N + ch * NCH: ct * N + (ch + 1) * NCH],
                        start=(ct == 0),
                        stop=(ct == DT - 1),
                    )
                nc.vector.tensor_scalar_add(
                    ob[:, ch * NCH:(ch + 1) * NCH], acc[:, :NCH],
                    bp_sb[:, mt8:mt8 + 1])
            if "X" not in mode:
                nc.sync.dma_start(out=out[mt8 * P:(mt8 + 1) * P, :], in_=ob[:])

    nc.compile()
    return nc


_NC_CACHE = {}


def _get_program(reps: int = 1, mode: str = ""):
    key = (reps, mode)
    if key not in _NC_CACHE:
        _NC_CACHE[key] = build_program(reps, mode)
    return _NC_CACHE[key]


def _tile_w(wT_slice):
    """(C, CL) weight slice -> pre-tiled (P, CT*CL) SBUF image."""
    c, cl = wT_slice.shape
    return np.ascontiguousarray(
        wT_slice.reshape(c // P, P, cl).transpose(1, 0, 2).reshape(P, -1))


def make_in_maps(query, key, value, Wq, Wk, Wv, Wp, bp):
    query = np.asarray(query, dtype=np.float32)
    key = np.asarray(key, dtype=np.float32)
    value = np.asarray(value, dtype=np.float32)
    Wq = np.asarray(Wq, dtype=np.float32)
    Wk = np.asarray(Wk, dtype=np.float32)
    Wv = np.asarray(Wv, dtype=np.float32)
    Wp = np.asarray(Wp, dtype=np.float32)
    bp = np.asarray(bp, dtype=np.float32)

    wqT = np.ascontiguousarray(Wq.T) * np.float32(SCALE)  # (C, C)
    wkT = np.ascontiguousarray(Wk.T)
    wvT = np.ascontiguousarray(Wv.T)
    wpT = np.ascontiguousarray(Wp.T)                      # (C, C)
    zeros_bp = np.zeros_like(bp)

    in_maps = []
    for core in range(8):
        b, g = divmod(core, 2)
        sl = slice(g * CL, (g + 1) * CL)
        bpc = (bp if g == 0 else zeros_bp)
        in_maps.append({
            "qTin": np.ascontiguousarray(query[b].T),
            "kTin": np.ascontiguousarray(key[b].T),
            "vTin": np.ascontiguousarray(value[b].T),
            "wq": _tile_w(wqT[:, sl]),
            "wk": _tile_w(wkT[:, sl]),
            "wv": _tile_w(wvT[:, sl]),
            "wp": _tile_w(wpT[sl, :]).astype(ml_dtypes.bfloat16),
            "bp": np.ascontiguousarray(bpc.reshape(CT, P).T),
        })
    return in_maps


def combine_outputs(results):
    out = np.empty((B, N, C), dtype=np.float32)
    for b in range(B):
        part = results[2 * b]["out"] + results[2 * b + 1]["out"]  # (C, N)
        out[b] = part.T
    return out


def kernel(**inputs) -> np.ndarray:
    nc = _get_program()
    in_maps = make_in_maps(**inputs)
    res = run_bass_kernel_spmd(nc, in_maps, list(range(8)))
    return combine_outputs(res.results)


if __name__ == "__main__":
    nc = _get_program()
    print("program built ok")


# revision 18
# speedup vs baseline: 1.0365x; 1.0248x over previous
"""CrossAttention (B=4, N=M=2048, C=1024, H=16, D=64) on 8 TRN2 cores.

Sharding: core = 2*b + g  (b = batch 0..3, g = head-half 0..1, 8 heads each).
Each core computes attention for its 8 heads and a partial (full-width)
output projection over its 512 local channels; the host sums the two
partials per batch and transposes back.

Device layout notes:
  - All activations live transposed (channels on partitions) so every
    matmul has its contraction on the partition axis with no on-chip
    transposes.  The host feeds query/key/value pre-transposed and the
    weights pre-tiled to the exact SBUF layout (contiguous DMAs).
  - The whole attention phase runs in the PE's 64x128 row-tiled mode
    (2 concurrent half-array tiles T0/T8), with no mode switches:
      * scoresT per head pair: head 2j on tile (0,0), head 2j+1 on
        (64,0), writing the two 512-wide halves (different PSUM banks)
        of one [128,1024] tile -> one 1024-wide exp covers both heads.
      * AV is split over key halves: keys 0-63 of a tile on (0,0),
        keys 64-127 on (64,0), accumulated into separate lo/hi PSUM
        accumulators that a DVE add folds together at the end.
    Mixing 64-row scores with 128-row AV (the obvious layout) would
    drain the PE array on every row-tiling mode switch (~2 per key
    tile, ~512 per kernel).
  - softmax denominator comes free from a ones-column appended to V
    (65-col AV stationary; row 64 of the accumulator is sum_m exp(s)).
  - exp on the scalar engine (psum->sbuf, width 1024, shifted by -2 so
    bf16 p~ stays in range; softmax is shift-invariant).
  - Normalization: DVE lo+hi fold + reciprocal of the denominator row +
    GPSIMD partition-broadcast + DVE multiply into xT.
  - Projection order q, k, v lets the score/exp stream start while v
    still projects (scores only need qT/kT + psum).
"""

from contextlib import ExitStack

import ml_dtypes
import numpy as np

import concourse.bass as bass
import concourse.mybir as mybir
import concourse.tile as tile
from concourse import bacc, library_config
from concourse.bass_utils import run_bass_kernel_spmd

dt = mybir.dt
AF = mybir.ActivationFunctionType

# Problem dims (hardcoded; must match the harness inputs).
B, N, M, C, H = 4, 2048, 2048, 1024, 16
D = C // H            # 64
SCALE = D ** -0.5     # 0.125 (exact)
CL = C // 2           # 512 channels per core (8 heads)
HL = H // 2           # 8 local heads
P = 128
CT = C // P           # 8 input-channel tiles
DT = CL // P          # 4 local-channel tiles
MT = M // P           # 16 key tiles
NCH = 512             # psum bank width in fp32
NCHUNKS = N // NCH    # 4
EXPW = 1024           # exp width (2 psum banks)
VA = D + 1            # 65: v columns + ones column

F32 = dt.float32
F32R = dt.float32r
BF16 = dt.bfloat16


def build_program(reps: int = 1, mode: str = "") -> bass.Bass:
    """reps>1 repeats the whole body for timing (wall-time delta isolates
    device time from host/transfer overhead).

    mode flags (diagnostics): 'P' stop after projections, 'A' stop after
    attention, 'O' skip attention (zero xT), 'X' skip final output DMA."""
    nc = bacc.Bacc()
    nc.gpsimd.load_library(library_config.attn)

    qTin = nc.declare_dram_parameter("qTin", [C, N], F32R, isOutput=False)
    kTin = nc.declare_dram_parameter("kTin", [C, M], F32R, isOutput=False)
    vTin = nc.declare_dram_parameter("vTin", [C, M], F32R, isOutput=False)
    # weights arrive pre-tiled to SBUF layout: contiguous (P, x) DMAs
    wq = nc.declare_dram_parameter("wq", [P, CT * CL], F32R, isOutput=False)
    wk = nc.declare_dram_parameter("wk", [P, CT * CL], F32R, isOutput=False)
    wv = nc.declare_dram_parameter("wv", [P, CT * CL], F32R, isOutput=False)
    wp = nc.declare_dram_parameter("wp", [P, DT * C], BF16, isOutput=False)
    bp = nc.declare_dram_parameter("bp", [P, CT], F32, isOutput=False)
    out = nc.declare_dram_parameter("out", [C, N], F32, isOutput=True)

    with tile.TileContext(nc) as tc:
      for _rep in range(reps):
       with ExitStack() as ctx:
        # ---- persistent sbuf tensors -------------------------------------
        const_pool = ctx.enter_context(tc.tile_pool(name="consts", bufs=1))
        bp_sb = const_pool.tile([P, CT], F32)
        # q in two zero-padded copies: qTp[0] has head 2j's dims in rows
        # 0-63 and zeros in 64-127, qTp[1] the reverse.  Both score
        # matmuls of a head pair can then share the one full-128-row kT
        # stationary (the zero rows kill the other head's contraction),
        # which keeps the whole kernel in the PE's 128-row mode: no
        # row-tiling mode-switch drains, one ldweights per key tile.
        qTp = [const_pool.tile([P, DT * N], BF16, name=f"qTp{i}")
               for i in range(2)]
        kT_sb = const_pool.tile([P, DT * M], BF16)   # local k, transposed
        # v(+ones) per key tile and head: [P][mt][h][VA]
        va_sb = const_pool.tile([P, MT * HL * VA], BF16)
        xT_sb = const_pool.tile([P, DT * N], BF16)  # attention out, transposed

        nc.sync.dma_start(out=bp_sb[:], in_=bp[:, :])
        nc.vector.memset(qTp[0][D:P, :], 0.0)
        nc.vector.memset(qTp[1][0:D, :], 0.0)

        va3 = va_sb[:].rearrange("p (m h e) -> p m h e", m=MT, h=HL, e=VA)

        # ---- attention pools; pt opens before phase 1 so exp can run
        #      while v still projects (scores only need qT/kT + psum) ----
        pt_pool = ctx.enter_context(tc.tile_pool(name="pt", bufs=12))

        # ---- phase 1: projections (q, k halved; v with all tiles) ------
        with tc.tile_pool(name="inT", bufs=8) as in_pool, \
             tc.tile_pool(name="wcur", bufs=1) as w_pool, \
             tc.tile_pool(name="ps2", bufs=2, space="PSUM") as ps_pool:

            def load_ctile(src, ct):
                t = in_pool.tile([P, N], F32R, tag="inT", name=f"i{ct}")
                nc.sync.dma_start(out=t[:], in_=src[ct * P:(ct + 1) * P, :])
                return t

            # --- q and k projections (halved accumulation) ---
            for src, w_dram, dsts in ((qTin, wq, None), (kTin, wk, kT_sb)):
                w_sb = w_pool.tile([P, CT * CL], F32R, tag="w", name="w")
                nc.sync.dma_start(out=w_sb[:], in_=w_dram[:, :])
                for half in range(2):
                    tiles = [load_ctile(src, half * 4 + ci) for ci in range(4)]
                    for j in range(DT):
                        for ch in range(NCHUNKS):
                            acc = ps_pool.tile([P, EXPW], F32, tag="big",
                                               name="prj")
                            for ci in range(4):
                                ct = half * 4 + ci
                                nc.tensor.matmul(
                                    acc[:, :NCH],
                                    w_sb[:, ct * CL + j * P: ct * CL + (j + 1) * P],
                                    tiles[ci][:, ch * NCH:(ch + 1) * NCH],
                                    start=(ci == 0),
                                    stop=(ci == 3),
                                )
                            col = slice(j * N + ch * NCH,
                                        j * N + (ch + 1) * NCH)
                            if dsts is None:
                                # q: split the head pair into the padded
                                # copies (rows 0-63 -> qTp[0], 64-127 -> qTp[1])
                                parts = [(qTp[0][0:D, col], acc[0:D, :NCH]),
                                         (qTp[1][D:P, col], acc[D:P, :NCH])]
                            else:
                                parts = [(dsts[:, col], acc[:, :NCH])]
                            for dst, s_ in parts:
                                if half == 0:
                                    nc.vector.tensor_copy(dst, s_)
                                else:
                                    nc.vector.tensor_add(dst, s_, dst)

            # --- v projection (natural orientation, all c-tiles live) ---
            wv_sb = w_pool.tile([P, CT * CL], F32R, tag="w", name="w")
            nc.sync.dma_start(out=wv_sb[:], in_=wv[:, :])
            vtiles = [load_ctile(vTin, ct) for ct in range(CT)]
            for mt in range(MT):
                acc = ps_pool.tile([P, EXPW], F32, tag="big", name="prv")
                for ct in range(CT):
                    nc.tensor.matmul(
                        acc[:, :CL],
                        vtiles[ct][:, mt * P:(mt + 1) * P],
                        wv_sb[:, ct * CL:(ct + 1) * CL],
                        start=(ct == 0),
                        stop=(ct == CT - 1),
                    )
                blk = va3[:, mt, :, :]                    # (P, HL, VA)
                nc.vector.tensor_copy(
                    blk[:, :, :D],
                    acc[:, :CL].rearrange("p (h d) -> p h d", d=D),
                )
                nc.vector.memset(blk[:, :, D:VA], 1.0)

        if "P" in mode:
            continue

        # late pools reuse the freed phase-1 space
        sc_pool = ctx.enter_context(tc.tile_pool(name="sc", bufs=4, space="PSUM"))
        av_pool = ctx.enter_context(tc.tile_pool(name="av", bufs=4, space="PSUM"))
        wpx_pool = ctx.enter_context(tc.tile_pool(name="wpx", bufs=1))
        sm_pool = ctx.enter_context(tc.tile_pool(name="sm", bufs=4))
        ob_pool = ctx.enter_context(tc.tile_pool(name="ob", bufs=3))
        wp_sb = wpx_pool.tile([P, DT * C], BF16)
        nc.sync.dma_start(out=wp_sb[:], in_=wp[:, :])

        if "O" in mode:
            nc.vector.memset(xT_sb[:], 0.0)

        # ---- phase 2: attention, head pair x 512-query chunks, all in
        #      64x128 row-tiled mode ----------------------------------
        for j in ([] if "O" in mode else range(DT)):   # head pair j: 2j, 2j+1
            for nh in range(NCHUNKS):                  # 512-query chunk
                q0 = j * N + nh * NCH
                avs = [av_pool.tile([P, NCH], F32, tag="av", name=f"av{hh}")
                       for hh in range(2)]
                # AV emission lags the score/exp stream by AVLAG key tiles
                # so the PE never head-of-line blocks on "exp(t) done" —
                # by the time av(t) reaches the front of the PE queue the
                # activation has long retired and the PE->ACT->PE
                # semaphore round trip is fully hidden.
                AVLAG = 2
                pend = []

                def emit_av(pts, mt):
                    for hh in range(2):
                        nc.tensor.matmul(
                            avs[hh][:VA, :],
                            va3[:, mt, 2 * j + hh, :],
                            pts[hh][:, :],
                            start=(mt == 0),
                            stop=(mt == MT - 1),
                        )

                for mt in range(MT):
                    kst = kT_sb[:, j * M + mt * P: j * M + (mt + 1) * P]
                    scs = [sc_pool.tile([P, NCH], F32, tag="sc", name="sc")
                           for _ in range(2)]
                    for hh in range(2):
                        nc.tensor.matmul(
                            scs[hh][:, :], kst, qTp[hh][:, q0:q0 + NCH],
                            start=True, stop=True,
                        )
                    pts = []
                    for hh in range(2):
                        pt = pt_pool.tile([P, NCH], BF16, tag="pt", name="pt")
                        nc.scalar.activation(pt[:], scs[hh][:], AF.Exp)
                        pts.append(pt)
                    pend.append((pts, mt))
                    if len(pend) > AVLAG:
                        emit_av(*pend.pop(0))
                for p_ in pend:
                    emit_av(*p_)

                for hh in range(2):
                    roff = hh * D
                    rc = sm_pool.tile([1, NCH], F32, tag="rc", name="rc")
                    nc.vector.reciprocal(rc[:], avs[hh][D:VA, :])
                    bc = sm_pool.tile([D, NCH], F32, tag="bc", name="bc")
                    nc.gpsimd.partition_broadcast(bc[:], rc[:])
                    nc.vector.tensor_mul(
                        xT_sb[roff:roff + D, q0:q0 + NCH],
                        avs[hh][:D, :],
                        bc[:],
                    )

        if "A" in mode:
            continue
        # ---- phase 3: output projection (partial over local channels) ---
        for mt8 in range(CT):
            ob = ob_pool.tile([P, N], F32, tag="ob", name="ob")
            for ch in range(NCHUNKS):
                acc = sc_pool.tile([P, NCH], F32, tag="sc", name="po")
                for ct in range(DT):
                    nc.tensor.matmul(
                        acc[:, :NCH],
                        wp_sb[:, ct * C + mt8 * P: ct * C + (mt8 + 1) * P],
                        xT_sb[:, ct * N + ch * NCH: ct * N + (ch + 1) * NCH],
                        start=(ct == 0),
                        stop=(ct == DT - 1),
                    )
                nc.vector.tensor_scalar_add(
                    ob[:, ch * NCH:(ch + 1) * NCH], acc[:, :NCH],
                    bp_sb[:, mt8:mt8 + 1])
            if "X" not in mode:
                nc.sync.dma_start(out=out[mt8 * P:(mt8 + 1) * P, :], in_=ob[:])

    nc.compile()
    return nc


_NC_CACHE = {}


def _get_program(reps: int = 1, mode: str = ""):
    key = (reps, mode)
    if key not in _NC_CACHE:
        _NC_CACHE[key] = build_program(reps, mode)
    return _NC_CACHE[key]


def _tile_w(wT_slice):
    """(C, CL) weight slice -> pre-tiled (P, CT*CL) SBUF image."""
    c, cl = wT_slice.shape
    return np.ascontiguousarray(
        wT_slice.reshape(c // P, P, cl).transpose(1, 0, 2).reshape(P, -1))


def make_in_maps(query, key, value, Wq, Wk, Wv, Wp, bp):
    query = np.asarray(query, dtype=np.float32)
    key = np.asarray(key, dtype=np.float32)
    value = np.asarray(value, dtype=np.float32)
    Wq = np.asarray(Wq, dtype=np.float32)
    Wk = np.asarray(Wk, dtype=np.float32)
    Wv = np.asarray(Wv, dtype=np.float32)
    Wp = np.asarray(Wp, dtype=np.float32)
    bp = np.asarray(bp, dtype=np.float32)

    wqT = np.ascontiguousarray(Wq.T) * np.float32(SCALE)  # (C, C)
    wkT = np.ascontiguousarray(Wk.T)
    wvT = np.ascontiguousarray(Wv.T)
    wpT = np.ascontiguousarray(Wp.T)                      # (C, C)
    zeros_bp = np.zeros_like(bp)

    in_maps = []
    for core in range(8):
        b, g = divmod(core, 2)
        sl = slice(g * CL, (g + 1) * CL)
        bpc = (bp if g == 0 else zeros_bp)
        in_maps.append({
            "qTin": np.ascontiguousarray(query[b].T),
            "kTin": np.ascontiguousarray(key[b].T),
            "vTin": np.ascontiguousarray(value[b].T),
            "wq": _tile_w(wqT[:, sl]),
            "wk": _tile_w(wkT[:, sl]),
            "wv": _tile_w(wvT[:, sl]),
            "wp": _tile_w(wpT[sl, :]).astype(ml_dtypes.bfloat16),
            "bp": np.ascontiguousarray(bpc.reshape(CT, P).T),
        })
    return in_maps


def combine_outputs(results):
    out = np.empty((B, N, C), dtype=np.float32)
    for b in range(B):
        part = results[2 * b]["out"] + results[2 * b + 1]["out"]  # (C, N)
        out[b] = part.T
    return out


def kernel(**inputs) -> np.ndarray:
    nc = _get_program()
    in_maps = make_in_maps(**inputs)
    res = run_bass_kernel_spmd(nc, in_maps, list(range(8)))
    return combine_outputs(res.results)


if __name__ == "__main__":
    nc = _get_program()
    print("program built ok")


# revision 20
# speedup vs baseline: 1.0370x; 1.0004x over previous
"""CrossAttention (B=4, N=M=2048, C=1024, H=16, D=64) on 8 TRN2 cores.

Sharding: core = 2*b + g  (b = batch 0..3, g = head-half 0..1, 8 heads each).
Each core computes attention for its 8 heads and a partial (full-width)
output projection over its 512 local channels; the host sums the two
partials per batch and transposes back.

Device layout notes:
  - All activations live transposed (channels on partitions) so every
    matmul has its contraction on the partition axis with no on-chip
    transposes.  The host feeds query/key/value pre-transposed and the
    weights pre-tiled to the exact SBUF layout (contiguous DMAs).
  - The whole attention phase runs in the PE's 64x128 row-tiled mode
    (2 concurrent half-array tiles T0/T8), with no mode switches:
      * scoresT per head pair: head 2j on tile (0,0), head 2j+1 on
        (64,0), writing the two 512-wide halves (different PSUM banks)
        of one [128,1024] tile -> one 1024-wide exp covers both heads.
      * AV is split over key halves: keys 0-63 of a tile on (0,0),
        keys 64-127 on (64,0), accumulated into separate lo/hi PSUM
        accumulators that a DVE add folds together at the end.
    Mixing 64-row scores with 128-row AV (the obvious layout) would
    drain the PE array on every row-tiling mode switch (~2 per key
    tile, ~512 per kernel).
  - softmax denominator comes free from a ones-column appended to V
    (65-col AV stationary; row 64 of the accumulator is sum_m exp(s)).
  - exp on the scalar engine (psum->sbuf, width 1024, shifted by -2 so
    bf16 p~ stays in range; softmax is shift-invariant).
  - Normalization: DVE lo+hi fold + reciprocal of the denominator row +
    GPSIMD partition-broadcast + DVE multiply into xT.
  - Projection order q, k, v lets the score/exp stream start while v
    still projects (scores only need qT/kT + psum).
"""

from contextlib import ExitStack

import ml_dtypes
import numpy as np

import concourse.bass as bass
import concourse.mybir as mybir
import concourse.tile as tile
from concourse import bacc, library_config
from concourse.bass_utils import run_bass_kernel_spmd

dt = mybir.dt
AF = mybir.ActivationFunctionType

# Problem dims (hardcoded; must match the harness inputs).
B, N, M, C, H = 4, 2048, 2048, 1024, 16
D = C // H            # 64
SCALE = D ** -0.5     # 0.125 (exact)
CL = C // 2           # 512 channels per core (8 heads)
HL = H // 2           # 8 local heads
P = 128
CT = C // P           # 8 input-channel tiles
DT = CL // P          # 4 local-channel tiles
MT = M // P           # 16 key tiles
NCH = 512             # psum bank width in fp32
NCHUNKS = N // NCH    # 4
EXPW = 1024           # exp width (2 psum banks)
VA = D + 1            # 65: v columns + ones column

F32 = dt.float32
F32R = dt.float32r
BF16 = dt.bfloat16


def build_program(reps: int = 1, mode: str = "") -> bass.Bass:
    """reps>1 repeats the whole body for timing (wall-time delta isolates
    device time from host/transfer overhead).

    mode flags (diagnostics): 'P' stop after projections, 'A' stop after
    attention, 'O' skip attention (zero xT), 'X' skip final output DMA."""
    nc = bacc.Bacc()
    nc.gpsimd.load_library(library_config.attn)

    qTin = nc.declare_dram_parameter("qTin", [C, N], F32R, isOutput=False)
    kTin = nc.declare_dram_parameter("kTin", [C, M], F32R, isOutput=False)
    vTin = nc.declare_dram_parameter("vTin", [C, M], F32R, isOutput=False)
    # weights arrive pre-tiled to SBUF layout: contiguous (P, x) DMAs
    wq = nc.declare_dram_parameter("wq", [P, CT * CL], F32R, isOutput=False)
    wk = nc.declare_dram_parameter("wk", [P, CT * CL], F32R, isOutput=False)
    wv = nc.declare_dram_parameter("wv", [P, CT * CL], F32R, isOutput=False)
    wp = nc.declare_dram_parameter("wp", [P, DT * C], BF16, isOutput=False)
    bp = nc.declare_dram_parameter("bp", [P, CT], F32, isOutput=False)
    out = nc.declare_dram_parameter("out", [C, N], F32, isOutput=True)

    with tile.TileContext(nc) as tc:
      for _rep in range(reps):
       with ExitStack() as ctx:
        # ---- persistent sbuf tensors -------------------------------------
        const_pool = ctx.enter_context(tc.tile_pool(name="consts", bufs=1))
        bp_sb = const_pool.tile([P, CT], F32)
        # q in two zero-padded copies: qTp[0] has head 2j's dims in rows
        # 0-63 and zeros in 64-127, qTp[1] the reverse.  Both score
        # matmuls of a head pair can then share the one full-128-row kT
        # stationary (the zero rows kill the other head's contraction),
        # which keeps the whole kernel in the PE's 128-row mode: no
        # row-tiling mode-switch drains, one ldweights per key tile.
        qTp = [const_pool.tile([P, DT * N], BF16, name=f"qTp{i}")
               for i in range(2)]
        kT_sb = const_pool.tile([P, DT * M], BF16)   # local k, transposed
        # v(+ones) per key tile and head: [P][mt][h][VA]
        va_sb = const_pool.tile([P, MT * HL * VA], BF16)
        xT_sb = const_pool.tile([P, DT * N], BF16)  # attention out, transposed

        nc.sync.dma_start(out=bp_sb[:], in_=bp[:, :])
        nc.vector.memset(qTp[0][D:P, :], 0.0)
        nc.vector.memset(qTp[1][0:D, :], 0.0)

        va3 = va_sb[:].rearrange("p (m h e) -> p m h e", m=MT, h=HL, e=VA)

        # ---- attention pools; pt opens before phase 1 so exp can run
        #      while v still projects (scores only need qT/kT + psum) ----
        pt_pool = ctx.enter_context(tc.tile_pool(name="pt", bufs=12))

        # ---- phase 1: projections (q, k halved; v with all tiles) ------
        with tc.tile_pool(name="inT", bufs=8) as in_pool, \
             tc.tile_pool(name="wcur", bufs=1) as w_pool, \
             tc.tile_pool(name="ps2", bufs=2, space="PSUM") as ps_pool:

            def load_ctile(src, ct):
                t = in_pool.tile([P, N], F32R, tag="inT", name=f"i{ct}")
                nc.sync.dma_start(out=t[:], in_=src[ct * P:(ct + 1) * P, :])
                return t

            # --- q and k projections (halved accumulation) ---
            for src, w_dram, dsts in ((qTin, wq, None), (kTin, wk, kT_sb)):
                w_sb = w_pool.tile([P, CT * CL], F32R, tag="w", name="w")
                nc.sync.dma_start(out=w_sb[:], in_=w_dram[:, :])
                for half in range(2):
                    tiles = [load_ctile(src, half * 4 + ci) for ci in range(4)]
                    for j in range(DT):
                        for ch in range(NCHUNKS):
                            acc = ps_pool.tile([P, EXPW], F32, tag="big",
                                               name="prj")
                            for ci in range(4):
                                ct = half * 4 + ci
                                nc.tensor.matmul(
                                    acc[:, :NCH],
                                    w_sb[:, ct * CL + j * P: ct * CL + (j + 1) * P],
                                    tiles[ci][:, ch * NCH:(ch + 1) * NCH],
                                    start=(ci == 0),
                                    stop=(ci == 3),
                                )
                            col = slice(j * N + ch * NCH,
                                        j * N + (ch + 1) * NCH)
                            if dsts is None:
                                # q: split the head pair into the padded
                                # copies (rows 0-63 -> qTp[0], 64-127 -> qTp[1])
                                parts = [(qTp[0][0:D, col], acc[0:D, :NCH]),
                                         (qTp[1][D:P, col], acc[D:P, :NCH])]
                            else:
                                parts = [(dsts[:, col], acc[:, :NCH])]
                            for dst, s_ in parts:
                                if half == 0:
                                    nc.vector.tensor_copy(dst, s_)
                                else:
                                    nc.vector.tensor_add(dst, s_, dst)

            # --- v projection (natural orientation, all c-tiles live) ---
            wv_sb = w_pool.tile([P, CT * CL], F32R, tag="w", name="w")
            nc.sync.dma_start(out=wv_sb[:], in_=wv[:, :])
            vtiles = [load_ctile(vTin, ct) for ct in range(CT)]
            for mt in range(MT):
                acc = ps_pool.tile([P, EXPW], F32, tag="big", name="prv")
                for ct in range(CT):
                    nc.tensor.matmul(
                        acc[:, :CL],
                        vtiles[ct][:, mt * P:(mt + 1) * P],
                        wv_sb[:, ct * CL:(ct + 1) * CL],
                        start=(ct == 0),
                        stop=(ct == CT - 1),
                    )
                blk = va3[:, mt, :, :]                    # (P, HL, VA)
                nc.vector.tensor_copy(
                    blk[:, :, :D],
                    acc[:, :CL].rearrange("p (h d) -> p h d", d=D),
                )
                nc.vector.memset(blk[:, :, D:VA], 1.0)

        if "P" in mode:
            continue

        # late pools reuse the freed phase-1 space
        sc_pool = ctx.enter_context(tc.tile_pool(name="sc", bufs=2, space="PSUM"))
        av_pool = ctx.enter_context(tc.tile_pool(name="av", bufs=4, space="PSUM"))
        wpx_pool = ctx.enter_context(tc.tile_pool(name="wpx", bufs=1))
        sm_pool = ctx.enter_context(tc.tile_pool(name="sm", bufs=4))
        ob_pool = ctx.enter_context(tc.tile_pool(name="ob", bufs=3))
        wp_sb = wpx_pool.tile([P, DT * C], BF16)
        nc.sync.dma_start(out=wp_sb[:], in_=wp[:, :])

        if "O" in mode:
            nc.vector.memset(xT_sb[:], 0.0)

        # ---- phase 2: attention, head pair x 512-query chunks, all in
        #      64x128 row-tiled mode ----------------------------------
        for j in ([] if "O" in mode else range(DT)):   # head pair j: 2j, 2j+1
            for nh in range(NCHUNKS):                  # 512-query chunk
                q0 = j * N + nh * NCH
                avs = [av_pool.tile([P, NCH], F32, tag="av", name=f"av{hh}")
                       for hh in range(2)]
                # AV emission lags the score/exp stream by AVLAG key tiles
                # so the PE never head-of-line blocks on "exp(t) done" —
                # by the time av(t) reaches the front of the PE queue the
                # activation has long retired and the PE->ACT->PE
                # semaphore round trip is fully hidden.
                AVLAG = 2
                pend = []

                def emit_av(pts, mt):
                    for hh in range(2):
                        nc.tensor.matmul(
                            avs[hh][:VA, :],
                            va3[:, mt, 2 * j + hh, :],
                            pts[hh][:, :],
                            start=(mt == 0),
                            stop=(mt == MT - 1),
                        )

                for mt in range(MT):
                    kst = kT_sb[:, j * M + mt * P: j * M + (mt + 1) * P]
                    # both heads' scores into one [128,1024] tile (two
                    # PSUM banks) so a single wide exp covers the pair:
                    # halves the number of semaphore-waited ACT
                    # instructions on the critical stream
                    sc = sc_pool.tile([P, EXPW], F32, tag="sc", name="sc")
                    for hh in range(2):
                        nc.tensor.matmul(
                            sc[:, hh * NCH:(hh + 1) * NCH],
                            kst, qTp[hh][:, q0:q0 + NCH],
                            start=True, stop=True,
                        )
                    ptw = pt_pool.tile([P, EXPW], BF16, tag="pt", name="pt")
                    nc.scalar.activation(ptw[:], sc[:], AF.Exp)
                    pts = [ptw[:, :NCH], ptw[:, NCH:]]
                    pend.append((pts, mt))
                    if len(pend) > AVLAG:
                        emit_av(*pend.pop(0))
                for p_ in pend:
                    emit_av(*p_)

                for hh in range(2):
                    roff = hh * D
                    rc = sm_pool.tile([1, NCH], F32, tag="rc", name="rc")
                    nc.vector.reciprocal(rc[:], avs[hh][D:VA, :])
                    bc = sm_pool.tile([D, NCH], F32, tag="bc", name="bc")
                    nc.gpsimd.partition_broadcast(bc[:], rc[:])
                    nc.vector.tensor_mul(
                        xT_sb[roff:roff + D, q0:q0 + NCH],
                        avs[hh][:D, :],
                        bc[:],
                    )

        if "A" in mode:
            continue
        # ---- phase 3: output projection (partial over local channels) ---
        for mt8 in range(CT):
            ob = ob_pool.tile([P, N], F32, tag="ob", name="ob")
            for ch in range(NCHUNKS):
                acc = sc_pool.tile([P, EXPW], F32, tag="sc", name="po")
                for ct in range(DT):
                    nc.tensor.matmul(
                        acc[:, :NCH],
                        wp_sb[:, ct * C + mt8 * P: ct * C + (mt8 + 1) * P],
                        xT_sb[:, ct * N + ch * NCH: ct * N + (ch + 1) * NCH],
                        start=(ct == 0),
                        stop=(ct == DT - 1),
                    )
                nc.vector.tensor_scalar_add(
                    ob[:, ch * NCH:(ch + 1) * NCH], acc[:, :NCH],
                    bp_sb[:, mt8:mt8 + 1])
            if "X" not in mode:
                nc.sync.dma_start(out=out[mt8 * P:(mt8 + 1) * P, :], in_=ob[:])

    nc.compile()
    return nc


_NC_CACHE = {}


def _get_program(reps: int = 1, mode: str = ""):
    key = (reps, mode)
    if key not in _NC_CACHE:
        _NC_CACHE[key] = build_program(reps, mode)
    return _NC_CACHE[key]


def _tile_w(wT_slice):
    """(C, CL) weight slice -> pre-tiled (P, CT*CL) SBUF image."""
    c, cl = wT_slice.shape
    return np.ascontiguousarray(
        wT_slice.reshape(c // P, P, cl).transpose(1, 0, 2).reshape(P, -1))


def make_in_maps(query, key, value, Wq, Wk, Wv, Wp, bp):
    query = np.asarray(query, dtype=np.float32)
    key = np.asarray(key, dtype=np.float32)
    value = np.asarray(value, dtype=np.float32)
    Wq = np.asarray(Wq, dtype=np.float32)
    Wk = np.asarray(Wk, dtype=np.float32)
    Wv = np.asarray(Wv, dtype=np.float32)
    Wp = np.asarray(Wp, dtype=np.float32)
    bp = np.asarray(bp, dtype=np.float32)

    wqT = np.ascontiguousarray(Wq.T) * np.float32(SCALE)  # (C, C)
    wkT = np.ascontiguousarray(Wk.T)
    wvT = np.ascontiguousarray(Wv.T)
    wpT = np.ascontiguousarray(Wp.T)                      # (C, C)
    zeros_bp = np.zeros_like(bp)

    in_maps = []
    for core in range(8):
        b, g = divmod(core, 2)
        sl = slice(g * CL, (g + 1) * CL)
        bpc = (bp if g == 0 else zeros_bp)
        in_maps.append({
            "qTin": np.ascontiguousarray(query[b].T),
            "kTin": np.ascontiguousarray(key[b].T),
            "vTin": np.ascontiguousarray(value[b].T),
            "wq": _tile_w(wqT[:, sl]),
            "wk": _tile_w(wkT[:, sl]),
            "wv": _tile_w(wvT[:, sl]),
            "wp": _tile_w(wpT[sl, :]).astype(ml_dtypes.bfloat16),
            "bp": np.ascontiguousarray(bpc.reshape(CT, P).T),
        })
    return in_maps


def combine_outputs(results):
    out = np.empty((B, N, C), dtype=np.float32)
    for b in range(B):
        part = results[2 * b]["out"] + results[2 * b + 1]["out"]  # (C, N)
        out[b] = part.T
    return out


def kernel(**inputs) -> np.ndarray:
    nc = _get_program()
    in_maps = make_in_maps(**inputs)
    res = run_bass_kernel_spmd(nc, in_maps, list(range(8)))
    return combine_outputs(res.results)


if __name__ == "__main__":
    nc = _get_program()
    print("program built ok")
